# revision 15
# baseline (speedup 1.0000x reference)
"""CircleLossV2 Trainium2 kernel v4: symmetric-half computation.

Exploits s_ij = s_ji: each row tile rt (rows = col-tile T = rt+1 in local
coords) computes exp factors F only for col tiles [T, T+32] (self + gaps
1..32, 4224 cols) instead of all 64 tiles:
  * rowpart_i  = sum over gaps 0..31 of F_ij   (ACT exp accum, 4096 cols)
  * gap-32 tile F computed (exp, no accum) for the colsum only
  * colsums over gaps 1..32 (4096 cols) via PE ones-matmul accumulated in
    a PSUM quadrant layout ([128,1664] f32: quadrant q in {0,1,2} at
    partitions [32q,32q+32), covering local cols 256+1664q..), shipped to host.
Every row j then gets: total_j = rowpart_j + sum of colparts_j where
colparts come from tiles T = V-32..V-1 (V = j's tile): coverage is exactly
all 64 tiles, each pair's F computed once. Same-label pairs (distance <= 32
after the label sort) are zeroed by the eq-penalty inside each tile's
window, so they are excluded from BOTH rowparts and colparts. ln/softplus/
assembly on host.
"""

import sys
import threading

import numpy as np

if "/opt/trn_rl_repo" not in sys.path:
    sys.path.insert(0, "/opt/trn_rl_repo")

from contextlib import ExitStack

import concourse.bass as bass  # noqa: F401
import concourse.tile as tile
import concourse.mybir as mybir
from concourse import bacc
from concourse.bass_utils import run_bass_kernel_spmd

AF = mybir.ActivationFunctionType
AL = mybir.AluOpType
AX = mybir.AxisListType
F32 = mybir.dt.float32
FP16 = mybir.dt.float16
BF16 = mybir.dt.bfloat16

B = 8192
D = 128
NCORES = 8
RPC = 1024
RO = 128
NRT = 8
CH = 512
SLAB = 1024       # PSUM slab width (2 banks)
MMW = 4608        # matmul col range width per rt (9 chunks of 512)
EW = 4224         # clamp/square/exp width (33 tiles: self + gaps 1..32)
RW = 4096         # rowpart (accum) width (self + gaps 1..31)
CW = 4096         # colsum width (gaps 1..32), dump-rel [128, 128+CW)
QW = 1664         # colacc quadrant width (f32; 3 quadrants at partitions 0/32/64)
PEN = -100.0
WIN = 192
WOFF = 96
MAXCNT = 33

# square-pass engine by E-rel range: [start, end, engine]
SQ_SPLIT = [(0, 2048, "dve"), (2048, 4224, "act")]


def _build_tile_kernel(ctx, tc, eT_d, eq_d, rowp_d, sump_d, mp_d, colp_d):
    nc = tc.nc

    big = ctx.enter_context(tc.tile_pool(name="big", bufs=1))
    small = ctx.enter_context(tc.tile_pool(name="small", bufs=1))
    sqp = ctx.enter_context(tc.tile_pool(name="sqp", bufs=3))
    clp = ctx.enter_context(tc.tile_pool(name="clp", bufs=4))
    dmp = ctx.enter_context(tc.tile_pool(name="dmp", bufs=3))
    work = ctx.enter_context(tc.tile_pool(name="work", bufs=4))
    psmm = ctx.enter_context(tc.tile_pool(name="psmm", bufs=2, space="PSUM"))
    psca = ctx.enter_context(tc.tile_pool(name="psca", bufs=1, space="PSUM"))

    eT = big.tile([128, B], FP16, tag="eT")
    for g in range(4):
        nc.sync.dma_start(eT[:, g * 2048 : (g + 1) * 2048],
                          eT_d[:, g * 2048 : (g + 1) * 2048])
    eqall = small.tile([128, NRT * WIN], FP16, tag="eqall")
    nc.sync.dma_start(eqall[:], eq_d)

    cm16 = small.tile([128, 1], F32, tag="cm16")
    nc.gpsimd.memset(cm16[:], -16.0)
    cm1 = small.tile([128, 1], F32, tag="cm1")
    nc.gpsimd.memset(cm1[:], -1.0)
    ones = small.tile([128, 1], BF16, tag="ones")
    nc.gpsimd.memset(ones[:], 1.0)
    rowp = small.tile([128, NRT], F32, tag="rowp")
    rowpa = small.tile([128, NRT], F32, tag="rowpa")
    rowpb = small.tile([128, NRT], F32, tag="rowpb")
    rowpc = small.tile([128, NRT], F32, tag="rowpc")
    sump = small.tile([128, NRT], F32, tag="sump")
    mpall = small.tile([128, NRT], F32, tag="mpall")

    colacc = psca.tile([128, QW], F32, tag="colacc")
    nc.vector.memset(colacc[:], 0.0)

    def emit_exps(prt, psq):
        dump = dmp.tile([128, EW], BF16, tag="dump")
        nc.scalar.activation(dump[:, 0:1920], psq[:, 0:1920], AF.Exp,
                             bias=cm16[:], scale=256.0,
                             accum_out=rowpa[:, prt : prt + 1])
        nc.scalar.activation(dump[:, 1920:3968], psq[:, 1920:3968], AF.Exp,
                             bias=cm16[:], scale=256.0,
                             accum_out=rowpb[:, prt : prt + 1])
        nc.scalar.activation(dump[:, 3968:RW], psq[:, 3968:RW], AF.Exp,
                             bias=cm16[:], scale=256.0,
                             accum_out=rowpc[:, prt : prt + 1])
        nc.scalar.activation(dump[:, RW:EW], psq[:, RW:EW], AF.Exp,
                             bias=cm16[:], scale=256.0)
        return dump

    def emit_colsums(prt, dump):
        o0 = prt * 128
        pieces = []
        o = o0
        while o < o0 + CW:
            qd = o // QW
            nxt = min((qd + 1) * QW, o0 + CW,
                      qd * QW + ((o - qd * QW) // 512 + 1) * 512)
            pieces.append((o, nxt, qd))
            o = nxt
        for (a, b, qd) in pieces:
            da = a - o0 + 128
            nc.tensor.matmul(colacc[32 * qd : 32 * qd + 1,
                                    a - qd * QW : b - qd * QW],
                             ones[:], dump[:, da : da + (b - a)],
                             start=False, stop=True, skip_group_check=True)

    # Software-pipelined: iteration rt PRODUCES rt's sq (PE mm, DVE clamp/
    # square) and CONSUMES rt-1's sq (ACT exps, PE colsums). ACT runs
    # strictly in order (no exec lookahead), so its stream must only see
    # ops whose inputs are already complete: q2(rt) [needs slab0 only],
    # exps(rt-1) [complete], dpos(rt) [DVE smalls done by then], squares.
    prev = None
    for rt in range(NRT):
        m0 = rt * 128                      # mm range start (local col)
        e0 = m0 + 128                      # E range start (self block)
        lhs = eT[:, e0 : e0 + 128]
        sq = sqp.tile([128, EW], FP16, tag="sq")
        eqr = eqall[:, rt * WIN : (rt + 1) * WIN]

        nslab = 5                          # 4x1024 + 1x512
        prev_dump = None
        for sl in range(nslab):
            s0 = m0 + sl * SLAB            # local col of slab start
            w = SLAB if sl < 4 else 512
            ps = psmm.tile([128, SLAB], F32, tag="ps")
            for q in range(w // CH):
                nc.tensor.matmul(ps[:, q * CH : (q + 1) * CH], lhs,
                                 eT[:, s0 + q * CH : s0 + (q + 1) * CH],
                                 start=True, stop=True)
            # clamp: cl = max(s, -0.25) on the E-part of this slab
            lo = max(s0, e0)               # local col range of clamp
            hi = min(s0 + w, e0 + EW)
            cl = clp.tile([128, SLAB], FP16, tag=f"cl{sl % 4}")
            nc.vector.tensor_scalar(cl[:, 0 : hi - lo],
                                    ps[:, lo - s0 : hi - s0],
                                    -0.25, None, op0=AL.max)
            # squares for this slab's E-range, split by SQ_SPLIT engines
            for a, b, eng in SQ_SPLIT:
                ga, gb = max(a, lo - e0), min(b, hi - e0)   # E-rel overlap
                if ga >= gb:
                    continue
                sl_off = ga - (lo - e0)    # offset within cl
                if eng == "act":
                    nc.scalar.activation(sq[:, ga:gb],
                                         cl[:, sl_off : sl_off + gb - ga],
                                         AF.Square)
                else:
                    nc.vector.tensor_mul(sq[:, ga:gb],
                                         cl[:, sl_off : sl_off + gb - ga],
                                         cl[:, sl_off : sl_off + gb - ga])

            if sl == 0:
                # ---- pos branch: window = local [m0+96, m0+288) in slab0
                nc.vector.scalar_tensor_tensor(
                    sq[:, 0:160], eqr[:, 32:WIN], PEN, sq[:, 0:160],
                    op0=AL.mult, op1=AL.add)
                q2 = work.tile([128, WIN], FP16, tag="q2")
                nc.scalar.activation(q2[:], ps[:, 96 : 96 + WIN], AF.Square,
                                     bias=cm1[:], scale=1.0)
                qm = work.tile([128, WIN], FP16, tag="qm")
                nc.vector.scalar_tensor_tensor(qm[:], q2[:], 0.0, eqr,
                                               op0=AL.add, op1=AL.mult)
                nc.vector.reduce_max(mpall[:, rt : rt + 1], qm[:], axis=AX.X)
                bnp = work.tile([128, 1], F32, tag="bnp")
                nc.vector.tensor_scalar(bnp[:], mpall[:, rt : rt + 1], -256.0,
                                        None, op0=AL.mult)
                # drain previous rt's exps while this rt's slabs compute
                if prev is not None:
                    prev_dump = emit_exps(*prev)
                dpos = work.tile([128, WIN], FP16, tag="dpos")
                nc.scalar.activation(dpos[:], qm[:], AF.Exp, bias=bnp[:],
                                     scale=256.0,
                                     accum_out=sump[:, rt : rt + 1])

        if prev_dump is not None:
            emit_colsums(prev[0], prev_dump)
        prev = (rt, sq)

    last_dump = emit_exps(*prev)
    emit_colsums(prev[0], last_dump)

    # ---- evacuate colacc, ship raw partials ----
    nc.vector.tensor_add(rowp[:], rowpa[:], rowpb[:])
    nc.vector.tensor_add(rowp[:], rowp[:], rowpc[:])
    colp = small.tile([128, QW], F32, tag="colp")
    nc.scalar.activation(colp[:], colacc[:], AF.Copy)
    nc.sync.dma_start(colp_d, colp[:])
    nc.sync.dma_start(rowp_d, rowp[:])
    nc.sync.dma_start(sump_d, sump[:])
    nc.sync.dma_start(mp_d, mpall[:])


def build_nc():
    nc = bacc.Bacc("TRN2", target_bir_lowering=False, debug=False)
    eT_d = nc.dram_tensor("eT", [128, B], FP16, kind="ExternalInput").ap()
    eq_d = nc.dram_tensor("eq", [128, NRT * WIN], FP16,
                          kind="ExternalInput").ap()
    rowp_d = nc.dram_tensor("rowp", [128, NRT], F32,
                            kind="ExternalOutput").ap()
    sump_d = nc.dram_tensor("sump", [128, NRT], F32,
                            kind="ExternalOutput").ap()
    mp_d = nc.dram_tensor("mp", [128, NRT], F32, kind="ExternalOutput").ap()
    colp_d = nc.dram_tensor("colp", [128, QW], F32,
                            kind="ExternalOutput").ap()
    with tile.TileContext(nc) as tc:
        with ExitStack() as ctx:
            _build_tile_kernel(ctx, tc, eT_d, eq_d, rowp_d, sump_d, mp_d,
                               colp_d)
    nc.compile()
    return nc


_NC_LOCK = threading.Lock()
_NC_CACHE: list = []


def _get_nc():
    with _NC_LOCK:
        if not _NC_CACHE:
            _NC_CACHE.append(build_nc())
        return _NC_CACHE[0]


def make_in_maps(embeddings: np.ndarray, labels: np.ndarray):
    emb = np.ascontiguousarray(np.asarray(embeddings), dtype=np.float32)
    lab = np.asarray(labels)
    perm = np.argsort(lab, kind="stable")
    lab_s = lab[perm]
    emb_s = emb[perm]
    _, counts = np.unique(lab_s, return_counts=True)
    assert counts.max() <= MAXCNT, "pos window margin exceeded"
    cnt_per_row = np.repeat(counts, counts)
    valid = (cnt_per_row >= 2) & (cnt_per_row < B)

    nrm = np.maximum(np.linalg.norm(emb_s, axis=1, keepdims=True), 1e-12)
    e16 = (emb_s / nrm).astype(np.float16)

    in_maps = []
    for k in range(NCORES):
        shift = (k * RPC - RO) % B
        ek = np.roll(e16, -shift, axis=0)
        lk = np.roll(lab_s, -shift)
        eTk = np.ascontiguousarray(ek.T)
        eq = np.empty((128, NRT * WIN), dtype=np.float16)
        for rt in range(NRT):
            rl = lk[RO + rt * 128 : RO + (rt + 1) * 128]
            wl = lk[rt * 128 + WOFF : rt * 128 + WOFF + WIN]
            eq[:, rt * WIN : (rt + 1) * WIN] = (
                rl[:, None] == wl[None, :]).astype(np.float16)
        in_maps.append({"eT": eTk, "eq": eq})
    return in_maps, valid


def finish(results, valid):
    # Assemble global neg sums: rowparts + colparts, in sorted-row order.
    sumn = np.zeros(B, dtype=np.float64)
    sump = np.empty(B, dtype=np.float64)
    mp = np.empty(B, dtype=np.float64)
    for k, r in enumerate(results):
        rows = slice(k * RPC, (k + 1) * RPC)
        sumn[rows] += np.asarray(r["rowp"], np.float64).T.reshape(-1)
        sump[rows] = np.asarray(r["sump"], np.float64).T.reshape(-1)
        mp[rows] = np.asarray(r["mp"], np.float64).T.reshape(-1)
        colp = np.asarray(r["colp"], np.float64)  # [128, QW] quadrants
        shift = k * RPC - RO
        for qd in range(3):
            o_lo, o_hi = qd * QW, min((qd + 1) * QW, 7 * 128 + 4096)
            if o_lo >= o_hi:
                continue
            vals = colp[32 * qd, 0 : o_hi - o_lo]
            g = (np.arange(o_lo, o_hi) + 256 + shift) % B
            sumn[g] += vals
    z = np.log(np.maximum(sumn, 1e-300)) + np.log(np.maximum(sump, 1e-300)) \
        + 256.0 * mp - 16.0
    per_row = np.where(valid, np.logaddexp(0.0, z), 0.0)
    n_valid = max(int(valid.sum()), 1)
    return np.asarray(per_row.sum() / n_valid, dtype=np.float32)


def kernel(embeddings, labels):
    in_maps, valid = make_in_maps(embeddings, labels)
    nc = _get_nc()
    res = run_bass_kernel_spmd(nc, in_maps, core_ids=list(range(NCORES)))
    return finish(res.results, valid)
